# revision 36
# baseline (speedup 1.0000x reference)
"""Trainium2 Bass kernel for nn_Attention_73581379715274.

GQA attention layer (B=1, S=2048, D=2048, H=32, KVH=8, HD=64) with RoPE,
causal mask, per-head FFN (Linear(64,64)+SiLU), and output projection.

Sharding (8 NeuronCores):
  - Tensor-parallel over heads: core c owns q-heads 4c..4c+3 and kv-head c
    (column-parallel wq/wk/wv).
  - wo is sequence-parallel: per-head FFN outputs are exchanged with a
    single AllToAll (1 MB/core instead of 8 MB/core for an AllGather);
    each core then computes all 2048 output dims for its 256 positions
    with the full wo resident in SBUF.

On-chip layout: feature dims live on partitions (transposed), so QK^T
produces scores^T directly, the softmax denominator comes free from a
ones-augmented V column in the PV matmul, and no probability transposes
are needed. The QK->exp->PV chain is software-pipelined one k-tile ahead
so the PE never stalls on the Act engine's exp.
"""
import sys

sys.path.insert(0, "/opt/trn_rl_repo")

import numpy as np
import ml_dtypes

import concourse.bass as bass
import concourse.tile as tile
import concourse.mybir as mybir
from concourse import bacc
from concourse.bass_utils import run_bass_kernel_spmd
from concourse.masks import make_identity

BF16 = ml_dtypes.bfloat16

N_CORES = 8
B, S, D = 1, 2048, 2048
H, KVH = 32, 8
HD = 64
HPC = H // N_CORES          # 4 q-heads per core
ECOLS = HPC * HD            # 256 feature columns per core
PPC = S // N_CORES          # 256 output positions per core
S_CHUNK = 512
N_SCHUNK = S // S_CHUNK     # 4
KT = D // 128               # 16 k-tiles for the D contraction
ST = S // 128               # 16 sequence 128-tiles

_nc_cache = {}


def _pairswap_mask():
    m = []
    for i in range(0, 32, 2):
        m += [i + 1, i]
    return m


def build_nc(causal: bool, apply_mask_t: bool):
    f32, bf16 = mybir.dt.float32, mybir.dt.bfloat16
    nc = bacc.Bacc("TRN2", target_bir_lowering=False, debug=False,
                   num_devices=N_CORES)

    xT = nc.dram_tensor("xT", [D, S], bf16, kind="ExternalInput")
    # packed projection weights: [wq_c(256) | wk_c(64) | wv_c(64)]
    wp = nc.dram_tensor("wp", [D, 384], bf16, kind="ExternalInput")
    cos2 = nc.dram_tensor("cos2", [128, S], f32, kind="ExternalInput")
    sinsig = nc.dram_tensor("sinsig", [128, S], f32, kind="ExternalInput")
    fw_in = nc.dram_tensor("fw_in", [HD, HD], bf16, kind="ExternalInput")
    fb_in = nc.dram_tensor("fb_in", [HD, 1], f32, kind="ExternalInput")
    wo_full = nc.dram_tensor("wo_full", [D, D], bf16, kind="ExternalInput")
    use_maskt = apply_mask_t and not causal
    if use_maskt:
        maskT = nc.dram_tensor("maskT", [S, S], f32, kind="ExternalInput")
    out_c = nc.dram_tensor("out_c", [D, PPC], f32, kind="ExternalOutput")
    import os as _os
    debug_dumps = bool(int(_os.environ.get("KDBG", "0")))
    if debug_dumps:
        a2a_out_dump = nc.dram_tensor("a2a_out_dump", [D, PPC], bf16,
                                      kind="ExternalOutput")

    xT_r = xT.rearrange("(kt p) s -> p kt s", p=128)
    wo_r = wo_full.rearrange("(kt p) e -> p kt e", p=128)

    with tile.TileContext(nc) as tc:
        with (
            tc.tile_pool(name="persist", bufs=1) as persist,
            tc.tile_pool(name="dram", bufs=1, space="DRAM") as dram,
        ):
            # ---- persistent SBUF tensors ----
            qT = persist.tile([128, 2, S], bf16, name="qT")
            kkT = persist.tile([128, S], bf16, name="kkT")
            v_aug = persist.tile([128, ST, HD + 1], bf16, name="v_aug")
            # fw/fb duplicated onto both 64-partition bands so the tail FFN
            # can process two heads per 128-row k-tile
            fw2 = persist.tile([128, HD], bf16, name="fw2")
            fb2 = persist.tile([128, 1], f32, name="fb2")
            ones_col = persist.tile([1, HD], f32, name="ones_col")
            wo_sb = persist.tile([128, KT, D], bf16, name="wo_sb")
            ident = persist.tile([128, 128], f32, name="ident")
            make_identity(nc, ident[:])
            if causal:
                # multiplicative lower-triangular mask for the diagonal
                # 128-tiles: keep ex[kp, q'] iff q' >= kp. Built once; the
                # per-tile masking is then a cheap DVE multiply instead of
                # a Pool affine_select (1.1us Q7 launch each).
                tril2 = persist.tile([128, 2, 128], bf16, name="tril2")
                nc.vector.memset(tril2[:], 1.0)
                nc.gpsimd.affine_select(
                    tril2[:], tril2[:],
                    pattern=[[0, 2], [1, 128]],
                    compare_op=mybir.AluOpType.is_ge,
                    fill=0.0, base=0, channel_multiplier=-1)

            nc.sync.dma_start(fw2[0:HD, :], fw_in[:])
            nc.sync.dma_start(fb2[0:HD, :], fb_in[:])
            nc.vector.tensor_copy(fw2[HD:128, :], fw2[0:HD, :])
            nc.vector.tensor_copy(fb2[HD:128, :], fb2[0:HD, :])
            nc.vector.memset(ones_col[:], 1.0)
            nc.vector.memset(v_aug[:, :, HD:HD + 1], 1.0)

            import os as _os
            for _rep in range(int(_os.environ.get("KREP", "1"))):
              # ================= phase 1: projections + RoPE =================
              with (
                  tc.tile_pool(name="xt", bufs=1) as xt_pool,
                  tc.tile_pool(name="trig", bufs=1) as trig_pool,
                  tc.tile_pool(name="wp_pool", bufs=1) as wp_pool,
                  tc.tile_pool(name="pp_q", bufs=6, space="PSUM") as pp_q,
                  tc.tile_pool(name="vtr", bufs=2, space="PSUM") as vtr_ps,
                  tc.tile_pool(name="rope_a", bufs=3) as rope_a,
                  tc.tile_pool(name="rope_b", bufs=2) as rope_b,
                  tc.tile_pool(name="vtmp", bufs=1) as vtmp_pool,
              ):
                  # wp split per k-tile and interleaved with x so the first
                  # projection matmul starts ~2us in instead of waiting for
                  # a monolithic 1.5 MB wp DMA
                  wp_sb = wp_pool.tile([128, KT, 384], bf16, name="wp_sb")
                  wp_r = wp.rearrange("(kt p) j -> p kt j", p=128)
                  x_sb = xt_pool.tile([128, KT, S], bf16, name="x_sb")
                  for k in range(KT):
                      eng = nc.sync if k % 2 == 0 else nc.gpsimd
                      eng.dma_start(wp_sb[:, k, :], wp_r[:, k, :])
                      eng.dma_start(x_sb[:, k, :], xT_r[:, k, :])
                  cos_sb = trig_pool.tile([128, S], f32, name="cos_sb")
                  sin_sb = trig_pool.tile([128, S], f32, name="sin_sb")
                  nc.sync.dma_start(cos_sb[:], cos2[:])
                  nc.sync.dma_start(sin_sb[:], sinsig[:])

                  swap = _pairswap_mask()

                  # RoPE split in two stages so the PSUM chain slot frees as
                  # soon as its two readers (shuffle, cos-mul) and the g=2
                  # vt copy are done; the m2/add/transpose work trails
                  # without holding PSUM, unblocking the next batch's chains.
                  def rope_stage_a(ps, ci, g):
                      sl = bass.ts(ci, S_CHUNK)
                      np_rope = 128 if g < 2 else HD
                      sw = rope_a.tile([128, S_CHUNK], f32, name="sw",
                                       tag="sw")
                      nc.vector.stream_shuffle(sw[0:np_rope, :],
                                               ps[0:np_rope, :], swap)
                      m1 = rope_a.tile([128, S_CHUNK], f32, name="m1",
                                       tag="m1")
                      nc.vector.tensor_mul(m1[0:np_rope, :],
                                           ps[0:np_rope, :],
                                           cos_sb[0:np_rope, sl])
                      vt = None
                      if g == 2:
                          vt = vtmp_pool.tile([64, S_CHUNK], f32,
                                              name="vt", tag="vt")
                          nc.scalar.copy(vt[:], ps[HD:128, :])
                      return sw, m1, vt

                  def rope_stage_b(ci, g, sw, m1, vt):
                      sl = bass.ts(ci, S_CHUNK)
                      np_rope = 128 if g < 2 else HD
                      m2 = rope_b.tile([128, S_CHUNK], f32, name="m2",
                                       tag="m2")
                      nc.gpsimd.tensor_mul(m2[0:np_rope, :],
                                           sw[0:np_rope, :],
                                           sin_sb[0:np_rope, sl])
                      if g < 2:
                          nc.vector.tensor_add(qT[:, g, sl], m1[:], m2[:])
                      else:
                          nc.vector.tensor_add(kkT[0:HD, sl],
                                               m1[0:HD, :], m2[0:HD, :])
                          # duplicate roped k into rows 64:128 for the
                          # row-tiled two-head QK matmuls
                          nc.vector.tensor_copy(kkT[HD:128, sl],
                                                kkT[0:HD, sl])
                          for j in range(S_CHUNK // 128):
                              t_idx = ci * 4 + j
                              tp = vtr_ps.tile([128, 64], f32, name="vtp",
                                               tag="vtp")
                              nc.tensor.transpose(tp[:],
                                                  vt[:, bass.ts(j, 128)],
                                                  ident[0:HD, 0:HD])
                              nc.vector.tensor_copy(
                                  v_aug[:, t_idx, 0:HD], tp[:])

                  # k-outer over 6 concurrent PSUM chains: the PE starts as
                  # soon as the first x k-tile lands instead of waiting for
                  # the whole 8 MB x load.
                  chains = [(ci, g) for ci in range(N_SCHUNK)
                            for g in range(3)]
                  for b0, b1 in ((0, 6), (6, 9), (9, 12)):
                      batch = chains[b0:b1]
                      pss = {}
                      for (ci, g) in batch:
                          pss[(ci, g)] = pp_q.tile([128, S_CHUNK], f32,
                                                   name="projps",
                                                   tag="projps")
                      for k in range(KT):
                          for (ci, g) in batch:
                              nc.tensor.matmul(
                                  pss[(ci, g)][:],
                                  wp_sb[:, k, bass.ts(g, 128)],
                                  x_sb[:, k, bass.ts(ci, S_CHUNK)],
                                  start=(k == 0), stop=(k == KT - 1),
                              )
                      pend = []
                      for (ci, g) in batch:
                          pend.append((ci, g,
                                       *rope_stage_a(pss[(ci, g)], ci, g)))
                          if len(pend) >= 3:
                              rope_stage_b(*pend.pop(0))
                      for item in pend:
                          rope_stage_b(*item)

                  # full wo (needed only by the tail): issued at the end of
                  # phase 1 on the Act queue so the x load gets the full DMA
                  # bandwidth first; the 8 MB stream overlaps attention.
                  for k in range(KT):
                      nc.scalar.dma_start(wo_sb[:, k, :], wo_r[:, k, :])

              # ======= phase 2a: attention per chunk =======
              # split AllToAll: half A carries heads {0,1}, half B heads
              # {2,3}. A fires as soon as the last chunk's hp=0 results are
              # out, overlapping hp=1 attention and the tail's even-k-tile
              # FFN/wo work.
              a2a_in = [dram.tile([D // 2, PPC], bf16,
                                  name=f"a2a_in{_rep}_{h}",
                                  tag=f"a2a_in{_rep}_{h}") for h in range(2)]
              a2a_out = [dram.tile([D // 2, PPC], bf16,
                                   name=f"a2a_out{_rep}_{h}",
                                   tag=f"a2a_out{_rep}_{h}")
                         for h in range(2)]
              with (
                  tc.tile_pool(name="qk_ps", bufs=2, space="PSUM") as qk_ps,
                  tc.tile_pool(name="pv_ps", bufs=2, space="PSUM") as pv_ps,
                  tc.tile_pool(name="aux_ps", bufs=2, space="PSUM") as aux_ps,
                  tc.tile_pool(name="exp_sb", bufs=6) as exp_sb,
                  tc.tile_pool(name="attn_tmp", bufs=4) as attn_tmp,
                  tc.tile_pool(name="at_sb", bufs=6) as at_pool,
                  tc.tile_pool(name="mt_pool", bufs=4) as mt_pool,
              ):
                  for ci in range(N_SCHUNK):
                      sl = bass.ts(ci, S_CHUNK)
                      t_max = ci * 4 + 3 if causal else ST - 1
                      for hp in range(2):
                          pv = [pv_ps.tile([HD + 1, S_CHUNK], f32,
                                           name=f"pv{half}", tag="pv")
                                for half in range(2)]

                          def issue_qk(t):
                              kslice = bass.ts(t, 128)
                              dcol = max(t * 128 - ci * S_CHUNK, 0) \
                                  if causal else 0
                              w = S_CHUNK - dcol
                              qsl = bass.ds(ci * S_CHUNK + dcol, w)
                              mt = None
                              if use_maskt:
                                  mt = mt_pool.tile([128, S_CHUNK], f32,
                                                    name="mt", tag="mt")
                                  nc.sync.dma_start(mt[:], maskT[kslice, sl])
                              ps = qk_ps.tile([128, 2, S_CHUNK], f32,
                                              name="qk", tag="qk")
                              for half in range(2):
                                  nc.tensor.matmul(
                                      ps[:, half, dcol:],
                                      kkT[bass.ds(64 * half, 64), kslice],
                                      qT[bass.ds(64 * half, 64), hp, qsl],
                                      start=True, stop=True,
                                      tile_position=(64 * half, 0),
                                  )
                              return ps, mt, dcol

                          ps_c, mt_c, dcol_c = issue_qk(0)
                          for t in range(t_max + 1):
                              if t < t_max:
                                  ps_n, mt_n, dcol_n = issue_qk(t + 1)
                              if use_maskt:
                                  for half in range(2):
                                      nc.vector.scalar_tensor_tensor(
                                          ps_c[:, half, :], ps_c[:, half, :],
                                          0.125, mt_c[:],
                                          op0=mybir.AluOpType.mult,
                                          op1=mybir.AluOpType.add)
                              ex = exp_sb.tile([128, 2, S_CHUNK], bf16,
                                               name="ex", tag="exp")
                              nc.scalar.activation(
                                  ex[:, :, dcol_c:], ps_c[:, :, dcol_c:],
                                  mybir.ActivationFunctionType.Exp,
                                  bias=0.0, scale=1.0 if use_maskt else 0.125)
                              if causal and t * 128 >= ci * S_CHUNK:
                                  nc.vector.tensor_mul(
                                      ex[:, :, bass.ds(dcol_c, 128)],
                                      ex[:, :, bass.ds(dcol_c, 128)],
                                      tril2[:])
                              for half in range(2):
                                  nc.tensor.matmul(
                                      pv[half][:, dcol_c:], v_aug[:, t, :],
                                      ex[:, half, dcol_c:],
                                      start=(t == 0), stop=(t == t_max),
                                  )
                              if t < t_max:
                                  ps_c, mt_c, dcol_c = ps_n, mt_n, dcol_n

                          for half in range(2):
                              head = hp * 2 + half
                              # softmax denominator: the custom-DVE
                              # reciprocal needs an SBUF operand, so stage
                              # the PSUM ones-row through SBUF first
                              lrow = attn_tmp.tile([1, S_CHUNK], f32,
                                                   name="lrow", tag="lrow")
                              nc.vector.tensor_copy(lrow[:],
                                                    pv[half][HD:HD + 1, :])
                              rec = attn_tmp.tile([1, S_CHUNK], f32,
                                                  name="rec", tag="rec")
                              scr = attn_tmp.tile([1, S_CHUNK], f32,
                                                  name="scr", tag="scr")
                              nc.vector.reciprocal_approx_accurate(
                                  rec[:], lrow[:], scr[:])
                              # numerator to SBUF via Act (copy needs no
                              # act-table reload)
                              pvc = attn_tmp.tile([HD, S_CHUNK], f32,
                                                  name="pvc", tag="pvc")
                              nc.scalar.copy(pvc[:], pv[half][0:HD, :])
                              recb = aux_ps.tile([HD, S_CHUNK], f32,
                                                 name="recb", tag="aux")
                              nc.tensor.matmul(recb[:], ones_col[:], rec[:],
                                               start=True, stop=True)
                              # normalized pre-FFN attention output goes
                              # straight to the AllToAll send buffer, split
                              # by destination core (position block of 256);
                              # the per-head FFN+SiLU runs after the
                              # exchange so the Act engine keeps the Exp
                              # table loaded for the whole attention phase.
                              at = at_pool.tile([HD, S_CHUNK], bf16,
                                                name="at", tag="at")
                              nc.vector.tensor_mul(at[:], pvc[:], recb[:])
                              for j in range(2):
                                  d = 2 * ci + j
                                  nc.sync.dma_start(
                                      a2a_in[hp][
                                          bass.ds(128 * d + HD * half,
                                                  HD), :],
                                      at[:, bass.ts(j, PPC)])
                          if ci == N_SCHUNK - 1:
                              nc.gpsimd.collective_compute(
                                  "AllToAll", mybir.AluOpType.bypass,
                                  replica_groups=[list(range(N_CORES))],
                                  ins=[a2a_in[hp][:].opt()],
                                  outs=[a2a_out[hp][:].opt()],
                              )

              # ======= phase 2b: sequence-sharded FFN + wo =======
              if debug_dumps:
                  for h in range(2):
                      nc.sync.dma_start(
                          a2a_out_dump[bass.ds(1024 * h, 1024), :],
                          a2a_out[h][:])
              with (
                  tc.tile_pool(name="zt", bufs=1) as zt_pool,
                  tc.tile_pool(name="ffn_ps", bufs=2, space="PSUM") as ffn_ps,
                  tc.tile_pool(name="wo_ps", bufs=6, space="PSUM") as wo_ps,
                  tc.tile_pool(name="out_sb", bufs=3) as out_pool,
              ):
                  zT_sb = zt_pool.tile([128, KT, PPC], bf16, name="zT_sb")
                  zsT = zt_pool.tile([128, KT, PPC], bf16, name="zsT")
                  # half h k-tile kt2 holds global k-tile 2*kt2 + h
                  for h in range(2):
                      a2a_r = a2a_out[h][:].rearrange(
                          "(kt p) s -> p kt s", p=128)
                      for kt2 in range(KT // 2):
                          eng = nc.sync if kt2 % 2 == 0 else nc.gpsimd
                          eng.dma_start(zT_sb[:, 2 * kt2 + h, :],
                                        a2a_r[:, kt2, :])
                  # per-head FFN + SiLU, two heads per 128-row k-tile;
                  # even tiles (half A) first — they arrive earlier
                  k_order = [2 * i for i in range(KT // 2)] + \
                      [2 * i + 1 for i in range(KT // 2)]
                  for k in k_order:
                      zf = ffn_ps.tile([128, PPC], f32, name="zf", tag="zf")
                      nc.tensor.matmul(zf[0:HD, :], fw2[0:HD, :],
                                       zT_sb[0:HD, k, :],
                                       start=True, stop=True,
                                       tile_position=(0, 0))
                      nc.tensor.matmul(zf[HD:128, :], fw2[HD:128, :],
                                       zT_sb[HD:128, k, :],
                                       start=True, stop=True,
                                       tile_position=(HD, HD))
                      nc.scalar.activation(
                          zsT[:, k, :], zf[:],
                          mybir.ActivationFunctionType.Silu,
                          bias=fb2[:], scale=1.0)
                  # k-outer in j-groups of 6: the first group's accumulation
                  # tracks the FFN stage's zsT k-tiles as they appear
                  for jg in (range(0, 6), range(6, 12), range(12, KT)):
                      wops = {}
                      for j in jg:
                          wops[j] = wo_ps.tile([128, PPC], f32, name="wop",
                                               tag="wop")
                      for ki, k in enumerate(k_order):
                          for j in jg:
                              nc.tensor.matmul(
                                  wops[j][:], wo_sb[:, k, bass.ts(j, 128)],
                                  zsT[:, k, :],
                                  start=(ki == 0), stop=(ki == KT - 1),
                              )
                      for j in jg:
                          ob = out_pool.tile([128, PPC], f32, name="ob",
                                             tag="ob")
                          nc.scalar.copy(ob[:], wops[j][:])
                          nc.sync.dma_start(out_c[bass.ts(j, 128), :], ob[:])

    nc.finalize()
    return nc


def _host_prep(x, freqs_cos, freqs_sin, wq, wk, wv, wo, fw, fb):
    """Host-side layout prep (transposes, slicing, dtype casts only)."""
    x2 = np.asarray(x, dtype=np.float32).reshape(S, D)
    xT = np.ascontiguousarray(x2.T).astype(BF16)

    cosT = np.asarray(freqs_cos, np.float32).T          # [32, S]
    sinT = np.asarray(freqs_sin, np.float32).T
    cos64 = np.repeat(cosT, 2, axis=0)                  # [64, S]
    sin64 = np.repeat(sinT, 2, axis=0)
    sign = np.where((np.arange(HD) % 2) == 0, -1.0, 1.0).astype(np.float32)
    ss64 = sin64 * sign[:, None]
    cos2 = np.ascontiguousarray(np.tile(cos64, (2, 1)))     # [128, S]
    sinsig = np.ascontiguousarray(np.tile(ss64, (2, 1)))

    fwb = np.asarray(fw, np.float32).astype(BF16)           # [d, e] natural
    fbv = np.ascontiguousarray(np.asarray(fb, np.float32).reshape(HD, 1))

    wq_f = np.asarray(wq, np.float32)
    wk_f = np.asarray(wk, np.float32)
    wv_f = np.asarray(wv, np.float32)
    wo_b = np.ascontiguousarray(np.asarray(wo, np.float32)).astype(BF16)

    in_maps = []
    for c in range(N_CORES):
        wq_c = wq_f[:, c * ECOLS:(c + 1) * ECOLS]
        wk_c = wk_f[:, c * HD:(c + 1) * HD]
        wv_c = wv_f[:, c * HD:(c + 1) * HD]
        wpk = np.concatenate([wq_c, wk_c, wv_c], axis=1).astype(BF16)
        in_maps.append({
            "xT": xT, "wp": np.ascontiguousarray(wpk), "cos2": cos2,
            "sinsig": sinsig, "fw_in": fwb, "fb_in": fbv, "wo_full": wo_b,
        })
    return in_maps


def _classify_mask(mask):
    m = np.asarray(mask, np.float32)
    if not m.any():
        return "zeros"
    tril = np.tril(np.ones((S, S), dtype=bool))
    if np.all(m[tril] == 0.0) and np.all(m[~tril] <= -1e4):
        return "causal"
    return "generic"


def _host_reference(x, freqs_cos, freqs_sin, mask, wq, wk, wv, wo, fw, fb):
    """Exact numpy fallback (pathological inputs only): matches reference()."""
    xf = np.asarray(x, np.float32).reshape(S, D)
    q = (xf @ np.asarray(wq, np.float32)).reshape(S, H, HD)
    k = (xf @ np.asarray(wk, np.float32)).reshape(S, KVH, HD)
    v = (xf @ np.asarray(wv, np.float32)).reshape(S, KVH, HD)
    cos = np.asarray(freqs_cos, np.float32)[:, None, :]
    sin = np.asarray(freqs_sin, np.float32)[:, None, :]

    def rope(t):
        tr = t.reshape(S, t.shape[1], HD // 2, 2)
        re = tr[..., 0] * cos - tr[..., 1] * sin
        im = tr[..., 0] * sin + tr[..., 1] * cos
        return np.stack([re, im], axis=-1).reshape(t.shape)

    q = rope(q)
    k = rope(k)
    k = np.repeat(k, H // KVH, axis=1)
    v = np.repeat(v, H // KVH, axis=1)
    q = q.transpose(1, 0, 2)        # (H, S, HD)
    k = k.transpose(1, 0, 2)
    v = v.transpose(1, 0, 2)
    m = np.asarray(mask, np.float32)
    out = np.empty((H, S, HD), np.float32)
    for h in range(H):
        sc = (q[h] @ k[h].T) / np.sqrt(HD).astype(np.float32) + m
        sc = sc - sc.max(axis=1, keepdims=True)
        e = np.exp(sc)
        p = e / e.sum(axis=1, keepdims=True)
        out[h] = p @ v[h]
    z = out @ np.asarray(fw, np.float32) + np.asarray(fb, np.float32)
    z = z * (1.0 / (1.0 + np.exp(-np.clip(z, -80, 80))))
    z = z.transpose(1, 0, 2).reshape(S, H * HD)
    return (z @ np.asarray(wo, np.float32)).reshape(B, S, D).astype(np.float32)


def _score_bound(x, wq, wk):
    """Rigorous upper bound on |scores|/8 via per-head row norms (RoPE is a
    per-position rotation, so it preserves these norms)."""
    xf = np.asarray(x, np.float32).reshape(S, D)
    q = xf @ np.asarray(wq, np.float32)
    k = xf @ np.asarray(wk, np.float32)
    qn = np.linalg.norm(q.reshape(S, H, HD), axis=2).max(axis=0)
    kn = np.linalg.norm(k.reshape(S, KVH, HD), axis=2).max(axis=0)
    return float((qn.reshape(KVH, H // KVH) * kn[:, None]).max() / 8.0)


def kernel(**inputs):
    x = inputs["x"]
    mask = inputs["mask"]
    kind = _classify_mask(mask)
    causal = kind == "causal"
    apply_mask_t = kind == "generic"

    # Safety: the device fast path skips softmax max-subtraction (scores are
    # tiny for this model's data distribution). Guard rigorously; fall back
    # to an exact host computation for pathological inputs.
    bound = _score_bound(x, inputs["wq"], inputs["wk"])
    mf = np.asarray(mask, np.float32)
    if apply_mask_t:
        finite_max = float(mf.max())
        row_ceiling = mf.max(axis=1)
        ok = (bound + max(finite_max, 0.0) < 80.0) and \
            bool((row_ceiling - bound > -80.0).all())
    else:
        ok = bound < 80.0
    if not ok:
        return _host_reference(
            x, inputs["freqs_cos"], inputs["freqs_sin"], mask,
            inputs["wq"], inputs["wk"], inputs["wv"], inputs["wo"],
            inputs["fw"], inputs["fb"])

    key = (causal, apply_mask_t)
    if key not in _nc_cache:
        _nc_cache[key] = build_nc(causal, apply_mask_t)
    nc = _nc_cache[key]

    in_maps = _host_prep(
        x, inputs["freqs_cos"], inputs["freqs_sin"],
        inputs["wq"], inputs["wk"], inputs["wv"], inputs["wo"],
        inputs["fw"], inputs["fb"])
    if apply_mask_t:
        mT = np.ascontiguousarray(mf.T)
        for m in in_maps:
            m["maskT"] = mT

    res = run_bass_kernel_spmd(nc, in_maps, core_ids=list(range(N_CORES)))
    out = np.concatenate(
        [res.results[c]["out_c"].T for c in range(N_CORES)], axis=0)
    return np.ascontiguousarray(out).reshape(B, S, D).astype(np.float32)


# revision 40
# speedup vs baseline: 1.1078x; 1.1078x over previous
"""Trainium2 Bass kernel for nn_Attention_73581379715274.

GQA attention layer (B=1, S=2048, D=2048, H=32, KVH=8, HD=64) with RoPE,
causal mask, per-head FFN (Linear(64,64)+SiLU), and output projection.

Sharding (8 NeuronCores):
  - Tensor-parallel over heads: core c owns q-heads 4c..4c+3 and kv-head c
    (column-parallel wq/wk/wv).
  - wo is sequence-parallel: per-head FFN outputs are exchanged with a
    single AllToAll (1 MB/core instead of 8 MB/core for an AllGather);
    each core then computes all 2048 output dims for its 256 positions
    with the full wo resident in SBUF.

On-chip layout: feature dims live on partitions (transposed), so QK^T
produces scores^T directly, the softmax denominator comes free from a
ones-augmented V column in the PV matmul, and no probability transposes
are needed. The QK->exp->PV chain is software-pipelined one k-tile ahead
so the PE never stalls on the Act engine's exp.
"""
import sys

sys.path.insert(0, "/opt/trn_rl_repo")

import numpy as np
import ml_dtypes

import concourse.bass as bass
import concourse.tile as tile
import concourse.mybir as mybir
from concourse import bacc
from concourse.bass_utils import run_bass_kernel_spmd
from concourse.masks import make_identity

BF16 = ml_dtypes.bfloat16

N_CORES = 8
B, S, D = 1, 2048, 2048
H, KVH = 32, 8
HD = 64
HPC = H // N_CORES          # 4 q-heads per core
ECOLS = HPC * HD            # 256 feature columns per core
PPC = S // N_CORES          # 256 output positions per core
S_CHUNK = 512
N_SCHUNK = S // S_CHUNK     # 4
KT = D // 128               # 16 k-tiles for the D contraction
ST = S // 128               # 16 sequence 128-tiles

_nc_cache = {}


def _pairswap_mask():
    m = []
    for i in range(0, 32, 2):
        m += [i + 1, i]
    return m


def build_nc(causal: bool, apply_mask_t: bool):
    f32, bf16 = mybir.dt.float32, mybir.dt.bfloat16
    nc = bacc.Bacc("TRN2", target_bir_lowering=False, debug=False,
                   num_devices=N_CORES)

    xT = nc.dram_tensor("xT", [D, S], bf16, kind="ExternalInput")
    # packed projection weights: [wq_c(256) | wk_c(64) | wv_c(64)]
    wp = nc.dram_tensor("wp", [D, 384], bf16, kind="ExternalInput")
    cos2 = nc.dram_tensor("cos2", [128, S], f32, kind="ExternalInput")
    sinsig = nc.dram_tensor("sinsig", [128, S], f32, kind="ExternalInput")
    fw_in = nc.dram_tensor("fw_in", [HD, HD], bf16, kind="ExternalInput")
    fb_in = nc.dram_tensor("fb_in", [HD, 1], f32, kind="ExternalInput")
    wo_full = nc.dram_tensor("wo_full", [D, D], bf16, kind="ExternalInput")
    use_maskt = apply_mask_t and not causal
    if use_maskt:
        maskT = nc.dram_tensor("maskT", [S, S], f32, kind="ExternalInput")
    out_c = nc.dram_tensor("out_c", [D, PPC], f32, kind="ExternalOutput")
    import os as _os
    debug_dumps = bool(int(_os.environ.get("KDBG", "0")))
    if debug_dumps:
        a2a_out_dump = nc.dram_tensor("a2a_out_dump", [D, PPC], bf16,
                                      kind="ExternalOutput")

    xT_r = xT.rearrange("(kt p) s -> p kt s", p=128)
    wo_r = wo_full.rearrange("(kt p) e -> p kt e", p=128)

    with tile.TileContext(nc) as tc:
        with (
            tc.tile_pool(name="persist", bufs=1) as persist,
            tc.tile_pool(name="dram", bufs=1, space="DRAM") as dram,
        ):
            # ---- persistent SBUF tensors ----
            qT = persist.tile([128, 2, S], bf16, name="qT")
            kkT = persist.tile([128, S], bf16, name="kkT")
            v_aug = persist.tile([128, ST, HD + 1], bf16, name="v_aug")
            # fw/fb duplicated onto both 64-partition bands so the tail FFN
            # can process two heads per 128-row k-tile
            fw2 = persist.tile([128, HD], bf16, name="fw2")
            fb2 = persist.tile([128, 1], f32, name="fb2")
            ones_col = persist.tile([1, HD], f32, name="ones_col")
            wo_sb = persist.tile([128, KT, D], bf16, name="wo_sb")
            ident = persist.tile([128, 128], f32, name="ident")
            make_identity(nc, ident[:])
            if causal:
                # multiplicative lower-triangular mask for the diagonal
                # 128-tiles: keep ex[kp, q'] iff q' >= kp. Built once; the
                # per-tile masking is then a cheap DVE multiply instead of
                # a Pool affine_select (1.1us Q7 launch each).
                tril2 = persist.tile([128, 2, 128], bf16, name="tril2")
                nc.vector.memset(tril2[:], 1.0)
                nc.gpsimd.affine_select(
                    tril2[:], tril2[:],
                    pattern=[[0, 2], [1, 128]],
                    compare_op=mybir.AluOpType.is_ge,
                    fill=0.0, base=0, channel_multiplier=-1)

            nc.sync.dma_start(fw2[0:HD, :], fw_in[:])
            nc.sync.dma_start(fb2[0:HD, :], fb_in[:])
            nc.vector.tensor_copy(fw2[HD:128, :], fw2[0:HD, :])
            nc.vector.tensor_copy(fb2[HD:128, :], fb2[0:HD, :])
            nc.vector.memset(ones_col[:], 1.0)
            nc.vector.memset(v_aug[:, :, HD:HD + 1], 1.0)

            import os as _os
            for _rep in range(int(_os.environ.get("KREP", "1"))):
              # ================= phase 1: projections + RoPE =================
              with (
                  tc.tile_pool(name="xt", bufs=1) as xt_pool,
                  tc.tile_pool(name="trig", bufs=1) as trig_pool,
                  tc.tile_pool(name="wp_pool", bufs=1) as wp_pool,
                  tc.tile_pool(name="pp_q", bufs=6, space="PSUM") as pp_q,
                  tc.tile_pool(name="vtr", bufs=2, space="PSUM") as vtr_ps,
                  tc.tile_pool(name="rope_a", bufs=3) as rope_a,
                  tc.tile_pool(name="rope_b", bufs=2) as rope_b,
                  tc.tile_pool(name="vtmp", bufs=1) as vtmp_pool,
              ):
                  # wp split per k-tile and interleaved with x so the first
                  # projection matmul starts ~2us in instead of waiting for
                  # a monolithic 1.5 MB wp DMA
                  wp_sb = wp_pool.tile([128, KT, 384], bf16, name="wp_sb")
                  wp_r = wp.rearrange("(kt p) j -> p kt j", p=128)
                  x_sb = xt_pool.tile([128, KT, S], bf16, name="x_sb")
                  for k in range(KT):
                      eng = nc.sync if k % 2 == 0 else nc.gpsimd
                      eng.dma_start(wp_sb[:, k, :], wp_r[:, k, :])
                      eng.dma_start(x_sb[:, k, :], xT_r[:, k, :])
                  cos_sb = trig_pool.tile([128, S], f32, name="cos_sb")
                  sin_sb = trig_pool.tile([128, S], f32, name="sin_sb")
                  nc.sync.dma_start(cos_sb[:], cos2[:])
                  nc.sync.dma_start(sin_sb[:], sinsig[:])

                  swap = _pairswap_mask()

                  # RoPE split in two stages so the PSUM chain slot frees as
                  # soon as its two readers (shuffle, cos-mul) and the g=2
                  # vt copy are done; the m2/add/transpose work trails
                  # without holding PSUM, unblocking the next batch's chains.
                  def rope_stage_a(ps, ci, g):
                      sl = bass.ts(ci, S_CHUNK)
                      np_rope = 128 if g < 2 else HD
                      sw = rope_a.tile([128, S_CHUNK], f32, name="sw",
                                       tag="sw")
                      nc.vector.stream_shuffle(sw[0:np_rope, :],
                                               ps[0:np_rope, :], swap)
                      m1 = rope_a.tile([128, S_CHUNK], f32, name="m1",
                                       tag="m1")
                      nc.vector.tensor_mul(m1[0:np_rope, :],
                                           ps[0:np_rope, :],
                                           cos_sb[0:np_rope, sl])
                      vt = None
                      if g == 2:
                          vt = vtmp_pool.tile([64, S_CHUNK], f32,
                                              name="vt", tag="vt")
                          nc.scalar.copy(vt[:], ps[HD:128, :])
                      return sw, m1, vt

                  def rope_stage_b(ci, g, sw, m1, vt):
                      sl = bass.ts(ci, S_CHUNK)
                      np_rope = 128 if g < 2 else HD
                      m2 = rope_b.tile([128, S_CHUNK], f32, name="m2",
                                       tag="m2")
                      nc.gpsimd.tensor_mul(m2[0:np_rope, :],
                                           sw[0:np_rope, :],
                                           sin_sb[0:np_rope, sl])
                      if g < 2:
                          nc.vector.tensor_add(qT[:, g, sl], m1[:], m2[:])
                      else:
                          nc.vector.tensor_add(kkT[0:HD, sl],
                                               m1[0:HD, :], m2[0:HD, :])
                          # duplicate roped k into rows 64:128 for the
                          # row-tiled two-head QK matmuls
                          nc.vector.tensor_copy(kkT[HD:128, sl],
                                                kkT[0:HD, sl])
                          for j in range(S_CHUNK // 128):
                              t_idx = ci * 4 + j
                              tp = vtr_ps.tile([128, 64], f32, name="vtp",
                                               tag="vtp")
                              nc.tensor.transpose(tp[:],
                                                  vt[:, bass.ts(j, 128)],
                                                  ident[0:HD, 0:HD])
                              nc.vector.tensor_copy(
                                  v_aug[:, t_idx, 0:HD], tp[:])

                  # k-outer over 6 concurrent PSUM chains: the PE starts as
                  # soon as the first x k-tile lands instead of waiting for
                  # the whole 8 MB x load.
                  chains = [(ci, g) for ci in range(N_SCHUNK)
                            for g in range(3)]
                  for b0, b1 in ((0, 6), (6, 9), (9, 12)):
                      batch = chains[b0:b1]
                      pss = {}
                      for (ci, g) in batch:
                          pss[(ci, g)] = pp_q.tile([128, S_CHUNK], f32,
                                                   name="projps",
                                                   tag="projps")
                      for k in range(KT):
                          for (ci, g) in batch:
                              nc.tensor.matmul(
                                  pss[(ci, g)][:],
                                  wp_sb[:, k, bass.ts(g, 128)],
                                  x_sb[:, k, bass.ts(ci, S_CHUNK)],
                                  start=(k == 0), stop=(k == KT - 1),
                              )
                      pend = []
                      for (ci, g) in batch:
                          pend.append((ci, g,
                                       *rope_stage_a(pss[(ci, g)], ci, g)))
                          if len(pend) >= 3:
                              rope_stage_b(*pend.pop(0))
                      for item in pend:
                          rope_stage_b(*item)

                  # full wo (needed only by the tail): issued at the end of
                  # phase 1 on the Act queue so the x load gets the full DMA
                  # bandwidth first; the 8 MB stream overlaps attention.
                  for k in range(KT):
                      nc.scalar.dma_start(wo_sb[:, k, :], wo_r[:, k, :])

              # ======= phase 2a: attention per chunk =======
              # split AllToAll: half A carries heads {0,1}, half B heads
              # {2,3}. A fires as soon as the last chunk's hp=0 results are
              # out, overlapping hp=1 attention and the tail's even-k-tile
              # FFN/wo work.
              a2a_in = [dram.tile([D // 2, PPC], bf16,
                                  name=f"a2a_in{_rep}_{h}",
                                  tag=f"a2a_in{_rep}_{h}") for h in range(2)]
              a2a_out = [dram.tile([D // 2, PPC], bf16,
                                   name=f"a2a_out{_rep}_{h}",
                                   tag=f"a2a_out{_rep}_{h}")
                         for h in range(2)]
              with (
                  tc.tile_pool(name="qk_ps", bufs=2, space="PSUM") as qk_ps,
                  tc.tile_pool(name="pv_ps", bufs=2, space="PSUM") as pv_ps,
                  tc.tile_pool(name="aux_ps", bufs=2, space="PSUM") as aux_ps,
                  tc.tile_pool(name="exp_sb", bufs=6) as exp_sb,
                  tc.tile_pool(name="attn_tmp", bufs=4) as attn_tmp,
                  tc.tile_pool(name="at_sb", bufs=6) as at_pool,
                  tc.tile_pool(name="mt_pool", bufs=4) as mt_pool,
              ):
                  for ci in range(N_SCHUNK):
                      sl = bass.ts(ci, S_CHUNK)
                      t_max = ci * 4 + 3 if causal else ST - 1
                      for hp in range(2):
                          pv = [pv_ps.tile([HD + 1, S_CHUNK], f32,
                                           name=f"pv{half}", tag="pv")
                                for half in range(2)]

                          def issue_qk(t):
                              kslice = bass.ts(t, 128)
                              dcol = max(t * 128 - ci * S_CHUNK, 0) \
                                  if causal else 0
                              w = S_CHUNK - dcol
                              qsl = bass.ds(ci * S_CHUNK + dcol, w)
                              mt = None
                              if use_maskt:
                                  mt = mt_pool.tile([128, S_CHUNK], f32,
                                                    name="mt", tag="mt")
                                  nc.sync.dma_start(mt[:], maskT[kslice, sl])
                              ps = qk_ps.tile([128, 2, S_CHUNK], f32,
                                              name="qk", tag="qk")
                              for half in range(2):
                                  nc.tensor.matmul(
                                      ps[:, half, dcol:],
                                      kkT[bass.ds(64 * half, 64), kslice],
                                      qT[bass.ds(64 * half, 64), hp, qsl],
                                      start=True, stop=True,
                                      tile_position=(64 * half, 0),
                                  )
                              return ps, mt, dcol

                          ps_c, mt_c, dcol_c = issue_qk(0)
                          for t in range(t_max + 1):
                              if t < t_max:
                                  ps_n, mt_n, dcol_n = issue_qk(t + 1)
                              if use_maskt:
                                  for half in range(2):
                                      nc.vector.scalar_tensor_tensor(
                                          ps_c[:, half, :], ps_c[:, half, :],
                                          0.125, mt_c[:],
                                          op0=mybir.AluOpType.mult,
                                          op1=mybir.AluOpType.add)
                              ex = exp_sb.tile([128, 2, S_CHUNK], bf16,
                                               name="ex", tag="exp")
                              nc.scalar.activation(
                                  ex[:, :, dcol_c:], ps_c[:, :, dcol_c:],
                                  mybir.ActivationFunctionType.Exp,
                                  bias=0.0, scale=1.0 if use_maskt else 0.125)
                              if causal and t * 128 >= ci * S_CHUNK:
                                  nc.vector.tensor_mul(
                                      ex[:, :, bass.ds(dcol_c, 128)],
                                      ex[:, :, bass.ds(dcol_c, 128)],
                                      tril2[:])
                              for half in range(2):
                                  nc.tensor.matmul(
                                      pv[half][:, dcol_c:], v_aug[:, t, :],
                                      ex[:, half, dcol_c:],
                                      start=(t == 0), stop=(t == t_max),
                                  )
                              if t < t_max:
                                  ps_c, mt_c, dcol_c = ps_n, mt_n, dcol_n

                          for half in range(2):
                              head = hp * 2 + half
                              # softmax denominator: the custom-DVE
                              # reciprocal needs an SBUF operand, so stage
                              # the PSUM ones-row through SBUF first
                              lrow = attn_tmp.tile([1, S_CHUNK], f32,
                                                   name="lrow", tag="lrow")
                              nc.vector.tensor_copy(lrow[:],
                                                    pv[half][HD:HD + 1, :])
                              rec = attn_tmp.tile([1, S_CHUNK], f32,
                                                  name="rec", tag="rec")
                              scr = attn_tmp.tile([1, S_CHUNK], f32,
                                                  name="scr", tag="scr")
                              nc.vector.reciprocal_approx_accurate(
                                  rec[:], lrow[:], scr[:])
                              # numerator to SBUF via Act (copy needs no
                              # act-table reload)
                              pvc = attn_tmp.tile([HD, S_CHUNK], f32,
                                                  name="pvc", tag="pvc")
                              nc.scalar.copy(pvc[:], pv[half][0:HD, :])
                              recb = aux_ps.tile([HD, S_CHUNK], f32,
                                                 name="recb", tag="aux")
                              nc.tensor.matmul(recb[:], ones_col[:], rec[:],
                                               start=True, stop=True)
                              # normalized pre-FFN attention output goes
                              # straight to the AllToAll send buffer, split
                              # by destination core (position block of 256);
                              # the per-head FFN+SiLU runs after the
                              # exchange so the Act engine keeps the Exp
                              # table loaded for the whole attention phase.
                              at = at_pool.tile([HD, S_CHUNK], bf16,
                                                name="at", tag="at")
                              nc.vector.tensor_mul(at[:], pvc[:], recb[:])
                              for j in range(2):
                                  d = 2 * ci + j
                                  nc.sync.dma_start(
                                      a2a_in[hp][
                                          bass.ds(128 * d + HD * half,
                                                  HD), :],
                                      at[:, bass.ts(j, PPC)])
                          if ci == N_SCHUNK - 1:
                              nc.gpsimd.collective_compute(
                                  "AllToAll", mybir.AluOpType.bypass,
                                  replica_groups=[list(range(N_CORES))],
                                  ins=[a2a_in[hp][:].opt()],
                                  outs=[a2a_out[hp][:].opt()],
                              )

              # ======= phase 2b: sequence-sharded FFN + wo =======
              if debug_dumps:
                  for h in range(2):
                      nc.sync.dma_start(
                          a2a_out_dump[bass.ds(1024 * h, 1024), :],
                          a2a_out[h][:])
              with (
                  tc.tile_pool(name="zt", bufs=1) as zt_pool,
                  tc.tile_pool(name="ffn_ps", bufs=2, space="PSUM") as ffn_ps,
                  tc.tile_pool(name="wo_ps", bufs=6, space="PSUM") as wo_ps,
                  tc.tile_pool(name="out_sb", bufs=3) as out_pool,
              ):
                  zT_sb = zt_pool.tile([128, KT, PPC], bf16, name="zT_sb")
                  zsT = zt_pool.tile([128, KT, PPC], bf16, name="zsT")
                  # dependency-free warm matmuls on the resident wo tiles:
                  # they run during the AllToAll wait and keep the PE
                  # p-state at full clock for the FFN/wo tail
                  warm = ffn_ps.tile([128, PPC], f32, name="warm", tag="zf")
                  for _w in range(10):
                      nc.tensor.matmul(warm[:], wo_sb[:, 0, 0:128],
                                       wo_sb[:, 1, 0:PPC],
                                       start=True, stop=True)
                  # half h k-tile kt2 holds global k-tile 2*kt2 + h
                  for h in range(2):
                      a2a_r = a2a_out[h][:].rearrange(
                          "(kt p) s -> p kt s", p=128)
                      for kt2 in range(KT // 2):
                          eng = nc.sync if kt2 % 2 == 0 else nc.gpsimd
                          eng.dma_start(zT_sb[:, 2 * kt2 + h, :],
                                        a2a_r[:, kt2, :])
                  # per-head FFN + SiLU, two heads per 128-row k-tile;
                  # even tiles (half A) first — they arrive earlier
                  k_order = [2 * i for i in range(KT // 2)] + \
                      [2 * i + 1 for i in range(KT // 2)]
                  for k in k_order:
                      zf = ffn_ps.tile([128, PPC], f32, name="zf", tag="zf")
                      nc.tensor.matmul(zf[0:HD, :], fw2[0:HD, :],
                                       zT_sb[0:HD, k, :],
                                       start=True, stop=True,
                                       tile_position=(0, 0))
                      nc.tensor.matmul(zf[HD:128, :], fw2[HD:128, :],
                                       zT_sb[HD:128, k, :],
                                       start=True, stop=True,
                                       tile_position=(HD, HD))
                      nc.scalar.activation(
                          zsT[:, k, :], zf[:],
                          mybir.ActivationFunctionType.Silu,
                          bias=fb2[:], scale=1.0)
                  # k-outer in j-groups of 6: the first group's accumulation
                  # tracks the FFN stage's zsT k-tiles as they appear
                  for jg in (range(0, 6), range(6, 12), range(12, KT)):
                      wops = {}
                      for j in jg:
                          wops[j] = wo_ps.tile([128, PPC], f32, name="wop",
                                               tag="wop")
                      for ki, k in enumerate(k_order):
                          for j in jg:
                              nc.tensor.matmul(
                                  wops[j][:], wo_sb[:, k, bass.ts(j, 128)],
                                  zsT[:, k, :],
                                  start=(ki == 0), stop=(ki == KT - 1),
                              )
                      for j in jg:
                          ob = out_pool.tile([128, PPC], f32, name="ob",
                                             tag="ob")
                          nc.scalar.copy(ob[:], wops[j][:])
                          nc.sync.dma_start(out_c[bass.ts(j, 128), :], ob[:])

    nc.finalize()
    return nc


def _host_prep(x, freqs_cos, freqs_sin, wq, wk, wv, wo, fw, fb):
    """Host-side layout prep (transposes, slicing, dtype casts only)."""
    x2 = np.asarray(x, dtype=np.float32).reshape(S, D)
    xT = np.ascontiguousarray(x2.T).astype(BF16)

    cosT = np.asarray(freqs_cos, np.float32).T          # [32, S]
    sinT = np.asarray(freqs_sin, np.float32).T
    cos64 = np.repeat(cosT, 2, axis=0)                  # [64, S]
    sin64 = np.repeat(sinT, 2, axis=0)
    sign = np.where((np.arange(HD) % 2) == 0, -1.0, 1.0).astype(np.float32)
    ss64 = sin64 * sign[:, None]
    cos2 = np.ascontiguousarray(np.tile(cos64, (2, 1)))     # [128, S]
    sinsig = np.ascontiguousarray(np.tile(ss64, (2, 1)))

    fwb = np.asarray(fw, np.float32).astype(BF16)           # [d, e] natural
    fbv = np.ascontiguousarray(np.asarray(fb, np.float32).reshape(HD, 1))

    wq_f = np.asarray(wq, np.float32)
    wk_f = np.asarray(wk, np.float32)
    wv_f = np.asarray(wv, np.float32)
    wo_b = np.ascontiguousarray(np.asarray(wo, np.float32)).astype(BF16)

    in_maps = []
    for c in range(N_CORES):
        wq_c = wq_f[:, c * ECOLS:(c + 1) * ECOLS]
        wk_c = wk_f[:, c * HD:(c + 1) * HD]
        wv_c = wv_f[:, c * HD:(c + 1) * HD]
        wpk = np.concatenate([wq_c, wk_c, wv_c], axis=1).astype(BF16)
        in_maps.append({
            "xT": xT, "wp": np.ascontiguousarray(wpk), "cos2": cos2,
            "sinsig": sinsig, "fw_in": fwb, "fb_in": fbv, "wo_full": wo_b,
        })
    return in_maps


def _classify_mask(mask):
    m = np.asarray(mask, np.float32)
    if not m.any():
        return "zeros"
    tril = np.tril(np.ones((S, S), dtype=bool))
    if np.all(m[tril] == 0.0) and np.all(m[~tril] <= -1e4):
        return "causal"
    return "generic"


def _host_reference(x, freqs_cos, freqs_sin, mask, wq, wk, wv, wo, fw, fb):
    """Exact numpy fallback (pathological inputs only): matches reference()."""
    xf = np.asarray(x, np.float32).reshape(S, D)
    q = (xf @ np.asarray(wq, np.float32)).reshape(S, H, HD)
    k = (xf @ np.asarray(wk, np.float32)).reshape(S, KVH, HD)
    v = (xf @ np.asarray(wv, np.float32)).reshape(S, KVH, HD)
    cos = np.asarray(freqs_cos, np.float32)[:, None, :]
    sin = np.asarray(freqs_sin, np.float32)[:, None, :]

    def rope(t):
        tr = t.reshape(S, t.shape[1], HD // 2, 2)
        re = tr[..., 0] * cos - tr[..., 1] * sin
        im = tr[..., 0] * sin + tr[..., 1] * cos
        return np.stack([re, im], axis=-1).reshape(t.shape)

    q = rope(q)
    k = rope(k)
    k = np.repeat(k, H // KVH, axis=1)
    v = np.repeat(v, H // KVH, axis=1)
    q = q.transpose(1, 0, 2)        # (H, S, HD)
    k = k.transpose(1, 0, 2)
    v = v.transpose(1, 0, 2)
    m = np.asarray(mask, np.float32)
    out = np.empty((H, S, HD), np.float32)
    for h in range(H):
        sc = (q[h] @ k[h].T) / np.sqrt(HD).astype(np.float32) + m
        sc = sc - sc.max(axis=1, keepdims=True)
        e = np.exp(sc)
        p = e / e.sum(axis=1, keepdims=True)
        out[h] = p @ v[h]
    z = out @ np.asarray(fw, np.float32) + np.asarray(fb, np.float32)
    z = z * (1.0 / (1.0 + np.exp(-np.clip(z, -80, 80))))
    z = z.transpose(1, 0, 2).reshape(S, H * HD)
    return (z @ np.asarray(wo, np.float32)).reshape(B, S, D).astype(np.float32)


def _score_bound(x, wq, wk):
    """Rigorous upper bound on |scores|/8 via per-head row norms (RoPE is a
    per-position rotation, so it preserves these norms)."""
    xf = np.asarray(x, np.float32).reshape(S, D)
    q = xf @ np.asarray(wq, np.float32)
    k = xf @ np.asarray(wk, np.float32)
    qn = np.linalg.norm(q.reshape(S, H, HD), axis=2).max(axis=0)
    kn = np.linalg.norm(k.reshape(S, KVH, HD), axis=2).max(axis=0)
    return float((qn.reshape(KVH, H // KVH) * kn[:, None]).max() / 8.0)


def kernel(**inputs):
    x = inputs["x"]
    mask = inputs["mask"]
    kind = _classify_mask(mask)
    causal = kind == "causal"
    apply_mask_t = kind == "generic"

    # Safety: the device fast path skips softmax max-subtraction (scores are
    # tiny for this model's data distribution). Guard rigorously; fall back
    # to an exact host computation for pathological inputs.
    bound = _score_bound(x, inputs["wq"], inputs["wk"])
    mf = np.asarray(mask, np.float32)
    if apply_mask_t:
        finite_max = float(mf.max())
        row_ceiling = mf.max(axis=1)
        ok = (bound + max(finite_max, 0.0) < 80.0) and \
            bool((row_ceiling - bound > -80.0).all())
    else:
        ok = bound < 80.0
    if not ok:
        return _host_reference(
            x, inputs["freqs_cos"], inputs["freqs_sin"], mask,
            inputs["wq"], inputs["wk"], inputs["wv"], inputs["wo"],
            inputs["fw"], inputs["fb"])

    key = (causal, apply_mask_t)
    if key not in _nc_cache:
        _nc_cache[key] = build_nc(causal, apply_mask_t)
    nc = _nc_cache[key]

    in_maps = _host_prep(
        x, inputs["freqs_cos"], inputs["freqs_sin"],
        inputs["wq"], inputs["wk"], inputs["wv"], inputs["wo"],
        inputs["fw"], inputs["fb"])
    if apply_mask_t:
        mT = np.ascontiguousarray(mf.T)
        for m in in_maps:
            m["maskT"] = mT

    res = run_bass_kernel_spmd(nc, in_maps, core_ids=list(range(N_CORES)))
    out = np.concatenate(
        [res.results[c]["out_c"].T for c in range(N_CORES)], axis=0)
    return np.ascontiguousarray(out).reshape(B, S, D).astype(np.float32)
